# revision 4
# baseline (speedup 1.0000x reference)
"""Mixture-of-Experts (top-1 routing) Trainium2 kernel.

Strategy (expert-parallel with one overflow slot, per sharding hint):
 - Router (softmax / argmax / top-prob) evaluated on host — 8192x8, i.e.
   0.002% of the FLOPs; its cost is dispatch bookkeeping.
 - Core e owns expert e.  The first MT-1 m-tiles of a core hold tokens of
   its primary expert; the last m-tile is an overflow slot (own-expert
   overflow, or up to 128 tokens of one overloaded foreign expert, using
   the core's secondary weight tensor).
 - Each core runs a dense [C,1024] @ [1024,1024] GEMM on the TensorEngine
   with fp16 operands and fp32 PSUM accumulation (~4.5e-4 max rel err
   end-to-end).  PSUM eviction fuses the bias in a single DVE op per
   half-tile: out = (bias * top_p) + psum.

Pipeline layout (v3):
 - The Tile runtime exposes only 8 DMA completion-semaphore lanes and
   each HWDGE transfer pays ~1.5 us of issue+completion latency, so the
   input stream is packed into few, large, consumption-ordered
   transfers: per k-PAIR blocks, host-packed so every DMA is one
   contiguous HBM read.  The k0/k1 halves of the first pair are split so
   the very first matmul depends on a single 128 KB transfer.
 - Matmuls run in half-chunk passes: (4 m-tiles x 1 n-half) accumulated
   over all k.  A pass's 4 PSUM banks are evicted while the next pass
   (other n-half) runs on the other 4 banks, so evictions and output
   DMAs pipeline behind the TensorEngine instead of bunching up at the
   kernel tail.  Consumption order: chunk m0-3 n0 (xtA+wA pairs), n1
   (wB pairs), chunk m4+ n0/n1 (xtB pairs), overflow m-tile (w2 pairs).
 - 5 dummy matmuls on an *uninitialized* SBUF tile (values irrelevant —
   the target PSUM bank is overwritten with start=True before its first
   real use) run during the DMA head phase to open the PE HAM clock
   gate (1.2 -> 2.4 GHz); skipping the memset lets them start the
   moment the TileContext entry barrier releases.
 - Host scatters the compact per-core outputs back to token order
   (the "second all-to-all" / unshard step).
"""

import numpy as np

T, H, E = 8192, 1024, 8
N_CORES = 8
P = 128
KT = H // P          # 8 contraction tiles
KP = KT // 2         # 4 k-pairs
NFREE = 512          # matmul moving free dim (one PSUM bank of fp32)
NT = H // NFREE      # 2 output column tiles
CH = 4               # m-tiles per half-chunk (4m x 1n = 4 PSUM banks)

_BUILD_CACHE = {}


def _build(MT):
    """Build the SPMD Bass module for MT m-tiles per core (C = MT*128)."""
    import concourse.mybir as mybir
    import concourse.tile as tile
    from concourse import bacc

    C = MT * P
    A = min(CH, MT) * P  # xt columns consumed by the first chunk
    B = C - A            # remaining xt columns (chunks 1..)
    DT = mybir.dt.float16    # half-precision I/O, full-rate matmul
    F32 = mybir.dt.float32
    F16 = mybir.dt.float16
    ALU = mybir.AluOpType

    nc = bacc.Bacc("TRN2", target_bir_lowering=False, debug=False,
                   num_devices=N_CORES)

    # Host-packed contiguous k-pair blocks (pair j = k 2j and 2j+1):
    #   xtap[j] = [xt_2j[:, :A] | xt_2j+1[:, :A]]
    #   xtbp[j] = [xt_2j[:, A:] | xt_2j+1[:, A:]]
    #   wap[j]  = [w_2j[:, :NF] | w_2j+1[:, :NF]]   (n0 halves)
    #   wbp[j]  = [w_2j[:, NF:] | w_2j+1[:, NF:]]   (n1 halves)
    #   w2p[j]  = [w2_2j | w2_2j+1]                 (full, overflow expert)
    xtap_d = nc.dram_tensor("xtap", [KP, P, 2 * A], DT, kind="ExternalInput").ap()
    wap_d = nc.dram_tensor("wap", [KP, P, 2 * NFREE], DT, kind="ExternalInput").ap()
    wbp_d = nc.dram_tensor("wbp", [KP, P, 2 * NFREE], DT, kind="ExternalInput").ap()
    w2p_d = nc.dram_tensor("w2p", [KP, P, 2 * H], DT, kind="ExternalInput").ap()
    bias_d = nc.dram_tensor("bias", [P, H], F16, kind="ExternalInput").ap()
    bias2_d = nc.dram_tensor("bias2", [P, H], F16, kind="ExternalInput").ap()
    scale_d = nc.dram_tensor("scale", [P, MT], F32, kind="ExternalInput").ap()
    out_d = nc.dram_tensor("out", [MT, P, H], F16, kind="ExternalOutput").ap()
    if B:
        xtbp_d = nc.dram_tensor("xtbp", [KP, P, 2 * B], DT,
                                kind="ExternalInput").ap()

    m_chunks = [list(range(s, min(s + CH, MT))) for s in range(0, MT, CH)]

    with tile.TileContext(nc) as tc:
        with (
            tc.tile_pool(name="ins", bufs=1) as ins,
            tc.tile_pool(name="psum", bufs=1, space="PSUM") as psum_pool,
            tc.tile_pool(name="outp", bufs=2) as outp,
        ):
            xtA_sb = [ins.tile([P, 2 * A], DT, name=f"xtA{j}") for j in range(KP)]
            xtB_sb = [ins.tile([P, 2 * B], DT, name=f"xtB{j}") for j in range(KP)] if B else None
            wA_sb = [ins.tile([P, 2 * NFREE], DT, name=f"wA{j}") for j in range(KP)]
            wB_sb = [ins.tile([P, 2 * NFREE], DT, name=f"wB{j}") for j in range(KP)]
            w2_sb = [ins.tile([P, 2 * H], DT, name=f"w2_{j}") for j in range(KP)]
            bias_sb = ins.tile([P, H], F16, name="bias")
            bias2_sb = ins.tile([P, H], F16, name="bias2")
            scale_sb = ins.tile([P, MT], F32, name="scale")

            def xt_ap(k, m):
                j, kk = divmod(k, 2)
                if m < CH:
                    off = kk * A + m * P
                    return xtA_sb[j][:, off:off + P]
                off = kk * B + (m - CH) * P
                return xtB_sb[j][:, off:off + P]

            def w_ap(k, n, sec):
                j, kk = divmod(k, 2)
                if sec:
                    off = kk * H + n * NFREE
                    return w2_sb[j][:, off:off + NFREE]
                t = wA_sb[j] if n == 0 else wB_sb[j]
                return t[:, kk * NFREE:(kk + 1) * NFREE]

            # PE warm-up: dummy matmuls on a zeroed tile bridge the DMA
            # head phase so the HAM clock-gate (1.2 -> 2.4 GHz) starts
            # ramping at the earliest possible moment.  GpSimd reaches
            # its first user instruction soonest after the init barrier,
            # so the memset goes there, ahead of its SWDGE transfers.
            wz = ins.tile([P, P + NFREE], DT, name="wz")
            nc.gpsimd.memset(wz[:], 0)
            warm_ps = psum_pool.tile([P, NFREE], F32, name="ps3_1")
            for _ in range(6):
                nc.tensor.matmul(warm_ps[:], wz[:, :P], wz[:, P:],
                                 start=True, stop=True)

            # tiny scale/bias inputs ride the SWDGE GpSimd queue, keeping
            # both HWDGE queues free for the xt / w streams
            nc.gpsimd.dma_start(scale_sb[:], scale_d[:])
            nc.gpsimd.dma_start(bias_sb[:], bias_d[:])
            nc.gpsimd.dma_start(bias2_sb[:], bias2_d[:])

            # Input stream, in consumption order, alternating the two
            # HWDGE queues (FIFO per engine; 8 global completion lanes).
            # Pair 0 is split into its k0 / k1 halves so the first
            # matmul's dependency is a single 128 KB transfer.
            S, Q = nc.sync, nc.scalar
            S.dma_start(xtA_sb[0][:, :A], xtap_d[0][:, :A])
            Q.dma_start(wA_sb[0][:, :NFREE], wap_d[0][:, :NFREE])
            Q.dma_start(xtA_sb[0][:, A:], xtap_d[0][:, A:])
            S.dma_start(wA_sb[0][:, NFREE:], wap_d[0][:, NFREE:])
            for j in range(1, KP):
                qx, qw = (S, Q) if j % 2 == 1 else (Q, S)
                qx.dma_start(xtA_sb[j][:], xtap_d[j])
                qw.dma_start(wA_sb[j][:], wap_d[j])
            for j in range(KP):
                q = S if j % 2 == 0 else Q
                q.dma_start(wB_sb[j][:], wbp_d[j])
            if B:
                for j in range(KP):
                    q = Q if j % 2 == 0 else S
                    q.dma_start(xtB_sb[j][:], xtbp_d[j])
            # Secondary (overflow) weights: only consumed by the last
            # m-tile, ~75% through the stream; back of both queues.
            for j in range(KP):
                q = S if j % 2 == 0 else Q
                q.dma_start(w2_sb[j][:], w2p_d[j])

            # Half-chunk passes: (4m x 1n) accumulated over k, evicted
            # while the sibling n-half accumulates on the other banks.
            pass_idx = 0
            for chunk in m_chunks:
                for n in range(NT):
                    nsl = slice(n * NFREE, (n + 1) * NFREE)
                    ps = {}
                    for m in chunk:
                        ps[m] = psum_pool.tile([P, NFREE], F32,
                                               name=f"ps{m % CH}_{n}")
                    for k in range(KT):
                        for m in chunk:
                            nc.tensor.matmul(
                                ps[m][:],
                                xt_ap(k, m),
                                w_ap(k, n, sec=(m == MT - 1)),
                                start=(k == 0), stop=(k == KT - 1),
                            )
                    for mi, m in enumerate(chunk):
                        bsb = bias2_sb if m == MT - 1 else bias_sb
                        t = outp.tile([P, NFREE], F16, name=f"osb{m % CH}_{n}")
                        # out = bias * top_p + psum   (single DVE op)
                        nc.vector.scalar_tensor_tensor(
                            t[:], bsb[:, nsl],
                            scale_sb[:, m:m + 1], ps[m][:],
                            op0=ALU.mult, op1=ALU.add,
                        )
                        eng = S if (pass_idx + mi) % 2 == 0 else Q
                        eng.dma_start(out_d[m][:, nsl], t[:])
                    pass_idx += 1

    nc.compile()
    return nc


def _plan(counts):
    """Pick MT and the overflow assignment.

    Returns (MT, prim, ext, free) where each core's secondary (overflow)
    m-tile holds up to 128 tokens: its own expert's overflow beyond
    (MT-1)*128, or one foreign chunk of an overloaded expert.
    Feasibility: every expert's tokens beyond MT*128 must fit in
    128-token chunks on cores whose own expert fits in (MT-1)*128.
    """
    mt_hi = max(1, int(-(-counts.max() // P)))          # plain expert-parallel
    mt_lo = max(1, int(-(-(counts.sum() // E) // P)))
    for MT in range(mt_lo, mt_hi + 1):
        prim = (MT - 1) * P
        ext = [max(0, int(c) - MT * P) for c in counts]
        slots_needed = sum(-(-x // P) for x in ext)
        free = [e for e in range(E) if counts[e] <= prim]
        if slots_needed <= len(free):
            return MT, prim, ext, free
    MT = mt_hi
    prim = (MT - 1) * P
    return MT, prim, [0] * E, []


def kernel(input, gate, W, b):
    from concourse import bass_utils

    input = np.ascontiguousarray(input, dtype=np.float32)
    gate = np.ascontiguousarray(gate, dtype=np.float32)
    W = np.ascontiguousarray(W, dtype=np.float32)
    b = np.ascontiguousarray(b, dtype=np.float32)

    # ---- router (host): top-1 expert + its softmax probability ----
    g = gate.astype(np.float64)
    gm = g.max(axis=1, keepdims=True)
    top_p = (1.0 / np.exp(g - gm).sum(axis=1)).astype(np.float32)
    e_t = np.argmax(gate, axis=1)

    counts = np.bincount(e_t, minlength=E)
    order = np.argsort(e_t, kind="stable")
    starts = np.zeros(E + 1, dtype=np.int64)
    np.cumsum(counts, out=starts[1:])
    ids_of = [order[starts[e]:starts[e + 1]] for e in range(E)]

    MT, prim, ext, free = _plan(counts)
    C = MT * P
    A = min(CH, MT) * P
    B = C - A

    # Per-core token layout: primary expert tokens in cols [0, prim) and
    # own-overflow (up to 128) in the overflow slot; foreign chunks of
    # overloaded experts go to free cores' overflow slots.
    core_prim_ids = []      # ids in the primary region
    core_sec_ids = []       # ids in the overflow m-tile
    core_sec_expert = []
    for e in range(E):
        ids = ids_of[e]
        n_own_prim = min(len(ids), prim)
        n_own_sec = min(P, max(0, len(ids) - prim))
        core_prim_ids.append(ids[:n_own_prim])
        core_sec_ids.append(ids[n_own_prim:n_own_prim + n_own_sec])
        core_sec_expert.append(e)
    # distribute external overflow chunks to free cores
    free_iter = iter(free)
    for e in range(E):
        leftover = ids_of[e][prim + P:] if len(ids_of[e]) > prim + P else []
        o = 0
        while o < len(leftover):
            host = next(free_iter)
            chunk = leftover[o:o + P]
            core_sec_ids[host] = chunk
            core_sec_expert[host] = e
            o += P

    W16 = W.astype(np.float16)
    b16 = b.astype(np.float16)

    if MT not in _BUILD_CACHE:
        _BUILD_CACHE[MT] = _build(MT)
    nc = _BUILD_CACHE[MT]

    in_maps = []
    for e in range(E):
        pi, si, se = core_prim_ids[e], core_sec_ids[e], core_sec_expert[e]
        n_p, n_s = len(pi), len(si)

        xt = np.zeros((KT, P, C), dtype=np.float16)
        xtf = xt.reshape(H, C)
        if n_p:
            xtf[:, :n_p] = (input[pi].T * top_p[pi][None, :]).astype(np.float16)
        if n_s:
            xtf[:, prim:prim + n_s] = (input[si].T * top_p[si][None, :]).astype(np.float16)

        scale = np.zeros((MT, P), dtype=np.float32)
        sf = scale.reshape(C)
        sf[:n_p] = top_p[pi]
        sf[prim:prim + n_s] = top_p[si]
        scale = np.ascontiguousarray(scale.T)

        we = W16[e].reshape(KT, P, H)
        w2 = W16[se].reshape(KT, P, H)
        xtk = xt.reshape(KP, 2, P, C)
        wek = we.reshape(KP, 2, P, H)
        w2k = w2.reshape(KP, 2, P, H)

        def pair(src):  # [KP, 2, P, X] -> [KP, P, 2X]
            return np.ascontiguousarray(src.transpose(0, 2, 1, 3).reshape(
                KP, P, 2 * src.shape[3]))

        im = {
            "xtap": pair(xtk[:, :, :, :A]),
            "wap": pair(wek[:, :, :, :NFREE]),
            "wbp": pair(wek[:, :, :, NFREE:]),
            "w2p": pair(w2k),
            "bias": np.ascontiguousarray(np.broadcast_to(b16[e], (P, H))),
            "bias2": np.ascontiguousarray(np.broadcast_to(b16[se], (P, H))),
            "scale": scale,
        }
        if B:
            im["xtbp"] = pair(xtk[:, :, :, A:])
        in_maps.append(im)

    res = bass_utils.run_bass_kernel_spmd(nc, in_maps,
                                          core_ids=list(range(N_CORES)))

    out = np.empty((T, H), dtype=np.float32)
    for e in range(E):
        r = res.results[e]["out"].reshape(C, H)
        pi, si = core_prim_ids[e], core_sec_ids[e]
        if len(pi):
            out[pi] = r[:len(pi)].astype(np.float32)
        if len(si):
            out[si] = r[prim:prim + len(si)].astype(np.float32)
    return out


# revision 5
# speedup vs baseline: 1.0293x; 1.0293x over previous
"""Mixture-of-Experts (top-1 routing) Trainium2 kernel.

Strategy (expert-parallel with one overflow slot, per sharding hint):
 - Router (softmax / argmax / top-prob) evaluated on host — 8192x8, i.e.
   0.002% of the FLOPs; its cost is dispatch bookkeeping.
 - Core e owns expert e.  The first MT-1 m-tiles of a core hold tokens of
   its primary expert; the last m-tile is an overflow slot (own-expert
   overflow, or up to 128 tokens of one overloaded foreign expert, using
   the core's secondary weight tensor).
 - Each core runs a dense [C,1024] @ [1024,1024] GEMM on the TensorEngine
   with fp16 operands and fp32 PSUM accumulation (~4.5e-4 max rel err
   end-to-end).  PSUM eviction fuses the bias in a single DVE op per
   half-tile: out = (bias * top_p) + psum.

Pipeline layout (v5):
 - The PE HAM clock gate opens (1.2 -> 2.4 GHz) at the first fully-busy
   free-running ~3.4 us activity window: ANY PE idle gap before it opens
   pushes the unlock a whole window later, running everything at half
   clock meanwhile.  So the stream is engineered gap-free from the first
   warm-up matmul: 6 dummy matmuls bridge exactly to the arrival of the
   first real k-tile.
 - Each HWDGE transfer costs ~1.5 us issue+completion latency on top of
   its wire time, so early k-tiles ship in geometrically growing groups
   (k0 | k1 | k2-3 | k4-5 | k6-7 for the chunk0 columns and n0 weight
   halves) — arrivals outpace even full-clock consumption.  Later data
   (n1 weight halves, chunk1+ xt columns, overflow weights) ships in
   4-k blocks, host-packed so every DMA is one contiguous HBM read.
 - Matmuls run in half-chunk passes: (4 m-tiles x 1 n-half) accumulated
   over all k.  A pass's 4 PSUM banks are evicted while the next pass
   (other n-half) runs on the other 4 banks, so evictions and output
   DMAs pipeline behind the TensorEngine instead of bunching up at the
   kernel tail.
 - Host scatters the compact per-core outputs back to token order
   (the "second all-to-all" / unshard step).
"""

import numpy as np

T, H, E = 8192, 1024, 8
N_CORES = 8
P = 128
KT = H // P          # 8 contraction tiles
NFREE = 512          # matmul moving free dim (one PSUM bank of fp32)
NT = H // NFREE      # 2 output column tiles
CH = 4               # m-tiles per half-chunk (4m x 1n = 4 PSUM banks)

XGROUPS = [[0], [1], [2, 3], [4, 5], [6, 7]]   # early-k geometric groups
HGROUPS = [[0, 1, 2, 3], [4, 5, 6, 7]]         # bulk 4-k groups

_BUILD_CACHE = {}


def _build(MT):
    """Build the SPMD Bass module for MT m-tiles per core (C = MT*128)."""
    import concourse.mybir as mybir
    import concourse.tile as tile
    from concourse import bacc

    C = MT * P
    A = min(CH, MT) * P  # xt columns consumed by the first chunk
    B = C - A            # remaining xt columns (chunks 1..)
    DT = mybir.dt.float16    # half-precision I/O, full-rate matmul
    F32 = mybir.dt.float32
    F16 = mybir.dt.float16
    ALU = mybir.AluOpType

    nc = bacc.Bacc("TRN2", target_bir_lowering=False, debug=False,
                   num_devices=N_CORES)

    # Host-packed contiguous column-concat blocks per k-group.
    xta_d = [nc.dram_tensor(f"xta{gi}", [P, len(g) * A], DT,
                            kind="ExternalInput").ap()
             for gi, g in enumerate(XGROUPS)]
    wa_d = [nc.dram_tensor(f"wa{gi}", [P, len(g) * NFREE], DT,
                           kind="ExternalInput").ap()
            for gi, g in enumerate(XGROUPS)]
    wb_d = [nc.dram_tensor(f"wb{gi}", [P, len(g) * NFREE], DT,
                           kind="ExternalInput").ap()
            for gi, g in enumerate(HGROUPS)]
    w2_d = [nc.dram_tensor(f"w2_{gi}", [P, len(g) * H], DT,
                           kind="ExternalInput").ap()
            for gi, g in enumerate(HGROUPS)]
    bias_d = nc.dram_tensor("bias", [P, H], F16, kind="ExternalInput").ap()
    bias2_d = nc.dram_tensor("bias2", [P, H], F16, kind="ExternalInput").ap()
    scale_d = nc.dram_tensor("scale", [P, MT], F32, kind="ExternalInput").ap()
    out_d = nc.dram_tensor("out", [MT, P, H], F16, kind="ExternalOutput").ap()
    if B:
        xtb_d = [nc.dram_tensor(f"xtb{gi}", [P, len(g) * B], DT,
                                kind="ExternalInput").ap()
                 for gi, g in enumerate(HGROUPS)]

    # k -> (group index, index within group)
    xgi = {k: (gi, i) for gi, g in enumerate(XGROUPS) for i, k in enumerate(g)}
    hgi = {k: (gi, i) for gi, g in enumerate(HGROUPS) for i, k in enumerate(g)}

    m_chunks = [list(range(s, min(s + CH, MT))) for s in range(0, MT, CH)]

    with tile.TileContext(nc) as tc:
        with (
            tc.tile_pool(name="ins", bufs=1) as ins,
            tc.tile_pool(name="psum", bufs=1, space="PSUM") as psum_pool,
            tc.tile_pool(name="outp", bufs=2) as outp,
        ):
            xtA_sb = [ins.tile([P, len(g) * A], DT, name=f"xtA{gi}")
                      for gi, g in enumerate(XGROUPS)]
            wA_sb = [ins.tile([P, len(g) * NFREE], DT, name=f"wA{gi}")
                     for gi, g in enumerate(XGROUPS)]
            wB_sb = [ins.tile([P, len(g) * NFREE], DT, name=f"wB{gi}")
                     for gi, g in enumerate(HGROUPS)]
            w2_sb = [ins.tile([P, len(g) * H], DT, name=f"w2_{gi}")
                     for gi, g in enumerate(HGROUPS)]
            xtB_sb = ([ins.tile([P, len(g) * B], DT, name=f"xtB{gi}")
                       for gi, g in enumerate(HGROUPS)] if B else None)
            bias_sb = ins.tile([P, H], F16, name="bias")
            bias2_sb = ins.tile([P, H], F16, name="bias2")
            scale_sb = ins.tile([P, MT], F32, name="scale")

            def xt_ap(k, m):
                if m < CH:
                    gi, i = xgi[k]
                    off = i * A + m * P
                    return xtA_sb[gi][:, off:off + P]
                gi, i = hgi[k]
                off = i * B + (m - CH) * P
                return xtB_sb[gi][:, off:off + P]

            def w_ap(k, n, sec):
                if sec:
                    gi, i = hgi[k]
                    off = i * H + n * NFREE
                    return w2_sb[gi][:, off:off + NFREE]
                if n == 0:
                    gi, i = xgi[k]
                    return wA_sb[gi][:, i * NFREE:(i + 1) * NFREE]
                gi, i = hgi[k]
                return wB_sb[gi][:, i * NFREE:(i + 1) * NFREE]

            # PE warm-up: dummy matmuls on a zeroed tile bridge the DMA
            # head phase so the HAM clock-gate ramp starts at the
            # earliest possible moment.  GpSimd reaches its first user
            # instruction soonest after the init barrier, so the memset
            # goes there, ahead of its SWDGE transfers.
            wz = ins.tile([P, P + NFREE], DT, name="wz")
            nc.gpsimd.memset(wz[:], 0)
            warm_ps = psum_pool.tile([P, NFREE], F32, name="ps3_1")
            for _ in range(6):
                nc.tensor.matmul(warm_ps[:], wz[:, :P], wz[:, P:],
                                 start=True, stop=True)

            # tiny scale/bias inputs ride the SWDGE GpSimd queue, keeping
            # both HWDGE queues free for the xt / w streams
            nc.gpsimd.dma_start(scale_sb[:], scale_d[:])
            nc.gpsimd.dma_start(bias_sb[:], bias_d[:])
            nc.gpsimd.dma_start(bias2_sb[:], bias2_d[:])

            # Input stream, in consumption order, alternating the two
            # HWDGE queues (FIFO per engine; 8 global completion lanes).
            S, Q = nc.sync, nc.scalar
            for gi in range(len(XGROUPS)):
                qx, qw = (S, Q) if gi % 2 == 0 else (Q, S)
                qx.dma_start(xtA_sb[gi][:], xta_d[gi])
                qw.dma_start(wA_sb[gi][:], wa_d[gi])
            S.dma_start(wB_sb[0][:], wb_d[0])
            if B:
                Q.dma_start(xtB_sb[0][:], xtb_d[0])
                S.dma_start(xtB_sb[1][:], xtb_d[1])
            Q.dma_start(wB_sb[1][:], wb_d[1])
            # Secondary (overflow) weights: only consumed by the last
            # m-tile, ~75% through the stream; back of both queues.
            S.dma_start(w2_sb[0][:], w2_d[0])
            Q.dma_start(w2_sb[1][:], w2_d[1])

            # Half-chunk passes: (4m x 1n) accumulated over k, evicted
            # while the sibling n-half accumulates on the other banks.
            pass_idx = 0
            for chunk in m_chunks:
                for n in range(NT):
                    nsl = slice(n * NFREE, (n + 1) * NFREE)
                    ps = {}
                    for m in chunk:
                        ps[m] = psum_pool.tile([P, NFREE], F32,
                                               name=f"ps{m % CH}_{n}")
                    for k in range(KT):
                        for m in chunk:
                            nc.tensor.matmul(
                                ps[m][:],
                                xt_ap(k, m),
                                w_ap(k, n, sec=(m == MT - 1)),
                                start=(k == 0), stop=(k == KT - 1),
                            )
                    for mi, m in enumerate(chunk):
                        bsb = bias2_sb if m == MT - 1 else bias_sb
                        t = outp.tile([P, NFREE], F16, name=f"osb{m % CH}_{n}")
                        # out = bias * top_p + psum   (single DVE op)
                        nc.vector.scalar_tensor_tensor(
                            t[:], bsb[:, nsl],
                            scale_sb[:, m:m + 1], ps[m][:],
                            op0=ALU.mult, op1=ALU.add,
                        )
                        eng = S if (pass_idx + mi) % 2 == 0 else Q
                        eng.dma_start(out_d[m][:, nsl], t[:])
                    pass_idx += 1

    nc.compile()
    return nc


def _plan(counts):
    """Pick MT and the overflow assignment.

    Returns (MT, prim, ext, free) where each core's secondary (overflow)
    m-tile holds up to 128 tokens: its own expert's overflow beyond
    (MT-1)*128, or one foreign chunk of an overloaded expert.
    Feasibility: every expert's tokens beyond MT*128 must fit in
    128-token chunks on cores whose own expert fits in (MT-1)*128.
    """
    mt_hi = max(1, int(-(-counts.max() // P)))          # plain expert-parallel
    mt_lo = max(1, int(-(-(counts.sum() // E) // P)))
    for MT in range(mt_lo, mt_hi + 1):
        prim = (MT - 1) * P
        ext = [max(0, int(c) - MT * P) for c in counts]
        slots_needed = sum(-(-x // P) for x in ext)
        free = [e for e in range(E) if counts[e] <= prim]
        if slots_needed <= len(free):
            return MT, prim, ext, free
    MT = mt_hi
    prim = (MT - 1) * P
    return MT, prim, [0] * E, []


def kernel(input, gate, W, b):
    from concourse import bass_utils

    input = np.ascontiguousarray(input, dtype=np.float32)
    gate = np.ascontiguousarray(gate, dtype=np.float32)
    W = np.ascontiguousarray(W, dtype=np.float32)
    b = np.ascontiguousarray(b, dtype=np.float32)

    # ---- router (host): top-1 expert + its softmax probability ----
    g = gate.astype(np.float64)
    gm = g.max(axis=1, keepdims=True)
    top_p = (1.0 / np.exp(g - gm).sum(axis=1)).astype(np.float32)
    e_t = np.argmax(gate, axis=1)

    counts = np.bincount(e_t, minlength=E)
    order = np.argsort(e_t, kind="stable")
    starts = np.zeros(E + 1, dtype=np.int64)
    np.cumsum(counts, out=starts[1:])
    ids_of = [order[starts[e]:starts[e + 1]] for e in range(E)]

    MT, prim, ext, free = _plan(counts)
    C = MT * P
    A = min(CH, MT) * P
    B = C - A

    # Per-core token layout: primary expert tokens in cols [0, prim) and
    # own-overflow (up to 128) in the overflow slot; foreign chunks of
    # overloaded experts go to free cores' overflow slots.
    core_prim_ids = []      # ids in the primary region
    core_sec_ids = []       # ids in the overflow m-tile
    core_sec_expert = []
    for e in range(E):
        ids = ids_of[e]
        n_own_prim = min(len(ids), prim)
        n_own_sec = min(P, max(0, len(ids) - prim))
        core_prim_ids.append(ids[:n_own_prim])
        core_sec_ids.append(ids[n_own_prim:n_own_prim + n_own_sec])
        core_sec_expert.append(e)
    # distribute external overflow chunks to free cores
    free_iter = iter(free)
    for e in range(E):
        leftover = ids_of[e][prim + P:] if len(ids_of[e]) > prim + P else []
        o = 0
        while o < len(leftover):
            host = next(free_iter)
            chunk = leftover[o:o + P]
            core_sec_ids[host] = chunk
            core_sec_expert[host] = e
            o += P

    W16 = W.astype(np.float16)
    b16 = b.astype(np.float16)

    if MT not in _BUILD_CACHE:
        _BUILD_CACHE[MT] = _build(MT)
    nc = _BUILD_CACHE[MT]

    in_maps = []
    for e in range(E):
        pi, si, se = core_prim_ids[e], core_sec_ids[e], core_sec_expert[e]
        n_p, n_s = len(pi), len(si)

        xt = np.zeros((KT, P, C), dtype=np.float16)
        xtf = xt.reshape(H, C)
        if n_p:
            xtf[:, :n_p] = (input[pi].T * top_p[pi][None, :]).astype(np.float16)
        if n_s:
            xtf[:, prim:prim + n_s] = (input[si].T * top_p[si][None, :]).astype(np.float16)

        scale = np.zeros((MT, P), dtype=np.float32)
        sf = scale.reshape(C)
        sf[:n_p] = top_p[pi]
        sf[prim:prim + n_s] = top_p[si]
        scale = np.ascontiguousarray(scale.T)

        we = W16[e].reshape(KT, P, H)
        w2 = W16[se].reshape(KT, P, H)

        def cat(src, g, sl):  # column-concat of k-slices
            return np.ascontiguousarray(
                np.concatenate([src[k][:, sl] for k in g], axis=1))

        im = {
            "bias": np.ascontiguousarray(np.broadcast_to(b16[e], (P, H))),
            "bias2": np.ascontiguousarray(np.broadcast_to(b16[se], (P, H))),
            "scale": scale,
        }
        for gi, gk in enumerate(XGROUPS):
            im[f"xta{gi}"] = cat(xt, gk, slice(0, A))
            im[f"wa{gi}"] = cat(we, gk, slice(0, NFREE))
        for gi, gk in enumerate(HGROUPS):
            im[f"wb{gi}"] = cat(we, gk, slice(NFREE, H))
            im[f"w2_{gi}"] = cat(w2, gk, slice(0, H))
            if B:
                im[f"xtb{gi}"] = cat(xt, gk, slice(A, C))
        in_maps.append(im)

    res = bass_utils.run_bass_kernel_spmd(nc, in_maps,
                                          core_ids=list(range(N_CORES)))

    out = np.empty((T, H), dtype=np.float32)
    for e in range(E):
        r = res.results[e]["out"].reshape(C, H)
        pi, si = core_prim_ids[e], core_sec_ids[e]
        if len(pi):
            out[pi] = r[:len(pi)].astype(np.float32)
        if len(si):
            out[si] = r[prim:prim + len(si)].astype(np.float32)
    return out
